# revision 13
# baseline (speedup 1.0000x reference)
"""Trainium2 Bass kernel for nn_CrossAttention (B=2, S=2048, D=1024, H=16).

Sharding: 8 cores = 2 batches x 4 head-groups (4 heads each). The reference's
raw-memory reshape maps head h's output to contiguous output rows
[h*128, (h+1)*128) — each core independently produces a [512, 1024] block.

v2 design (vs. 293us baseline):
  - exp split 3-way: ACT does exact exp -> bf16; DVE and Pool run a 1-op
    Schraudolph fast-exp (n = int16(A*x + B), bitcast bf16). Validated
    pipeline max rel err 0.0083 vs the 2e-2 gate.
  - PV in bf16 (v tiles + exp output bf16, fp32 PSUM accumulate).
  - Software-pipelined S->exp->PV at g granularity (PV trails S by 1).
  - Per-(hp,j) epilogue: PSUM z^T pair -> bf16 SBUF copy (Pool) -> 4 packed
    PE transposes into one PSUM tile -> one reciprocal (DVE) + one
    free-dim-broadcast multiply (Pool) -> HBM bounce -> LayerNorm.
  - Non-PE work spread: biases/residual/scale on Pool, stats/rsqrt on DVE.
  - gamma/beta are identity in this problem: applied host-side only if
    they deviate (kernel() checks).
  - Input DMAs spread over 4 queues; K projection streams k-outer during
    the encT load; Q streams n-outer right after; V i-outer after that.
"""

import numpy as np

import concourse.bass as bass
import concourse.tile as tile
from concourse import bacc, mybir
from concourse.bass_utils import run_bass_kernel_spmd
from concourse.masks import make_identity

F32 = mybir.dt.float32
F32R = mybir.dt.float32r
BF16 = mybir.dt.bfloat16
I16 = mybir.dt.int16
AF = mybir.ActivationFunctionType
OP = mybir.AluOpType

B = 2
S = 2048          # sequence length (q and k)
D = 1024          # d_model
DH = 64           # head dim
HPC = 4           # heads per core
DG = HPC * DH     # 256 projection out-dims per core
ROWS = HPC * 128  # 512 output rows per core
N_CORES = 8
LN_EPS = 1e-5

# Schraudolph fast exp in bf16: exp(x) ~= bitcast_bf16(int16(A16*x + B16))
A16 = float(2 ** 7 / np.log(2))   # 184.6650...
B16 = float(127 * 128 - 2.87)

# exp engine split per (g, hl) slot: A=ACT exact, D=DVE fast, P=Pool fast
def _exp_pattern(na=17, nd=15, npp=0):
    counts = {"A": na, "D": nd, "P": npp}
    acc = {k: 0.0 for k in counts}
    pat = []
    for _ in range(32):
        for k in counts:
            acc[k] += counts[k] / 32.0
        pick = max(acc, key=lambda k: acc[k])
        acc[pick] -= 1.0
        pat.append(pick)
    return pat

EXP_PAT = _exp_pattern()


def build_bass(reps=1):
    nc = bacc.Bacc(None, target_bir_lowering=False, debug=False)

    encT_h = nc.declare_dram_parameter("encT", [D, S], F32R, isOutput=False)
    decT_h = nc.declare_dram_parameter("decT", [D, S], F32R, isOutput=False)
    wqT_h = nc.declare_dram_parameter("wqT", [D, DG], F32R, isOutput=False)
    wkT_h = nc.declare_dram_parameter("wkT", [D, DG], F32R, isOutput=False)
    wvT_h = nc.declare_dram_parameter("wvT", [D, DG], F32R, isOutput=False)
    bq_h = nc.declare_dram_parameter("bq", [DG], F32, isOutput=False)
    bk_h = nc.declare_dram_parameter("bk", [DG], F32, isOutput=False)
    bv_h = nc.declare_dram_parameter("bv", [DG], F32, isOutput=False)
    dec_blk_h = nc.declare_dram_parameter("dec_blk", [ROWS, D], F32, isOutput=False)
    out_h = nc.declare_dram_parameter("out", [ROWS, D], F32, isOutput=True)

    def bcast(ap, p=128):
        return bass.AP(tensor=ap.tensor, offset=ap.offset, ap=[[0, p]] + list(ap.ap))

    ENC_Q = ["sync", "scalar", "gpsimd", "sync", "scalar", "gpsimd", "sync", "scalar"]
    DEC_Q = ["gpsimd", "sync", "scalar", "gpsimd", "sync", "scalar", "gpsimd", "sync"]

    with tile.TileContext(nc) as tc:
        with (
            tc.tile_pool(name="consts", bufs=1) as consts,
            tc.tile_pool(name="kq", bufs=1) as kq,
            tc.tile_pool(name="vp", bufs=16) as vpool,
            tc.tile_pool(name="dram", bufs=1, space="DRAM") as dram,
        ):
            kT = [kq.tile([128, S], F32R, name=f"kT{m}", tag=f"kT{m}") for m in range(2)]
            qT = [kq.tile([128, S], F32R, name=f"qT{m}", tag=f"qT{m}") for m in range(2)]
            zd = dram.tile([HPC, S, DH], F32)

            # ---- constants (once) ----
            bq_sb = consts.tile([128, 2], F32)
            nc.gpsimd.dma_start(out=bq_sb, in_=bq_h[:].rearrange("(t p) -> p t", p=128))
            bk_sb = consts.tile([128, 2], F32)
            nc.gpsimd.dma_start(out=bk_sb, in_=bk_h[:].rearrange("(t p) -> p t", p=128))
            bv_b = consts.tile([128, DG], F32)
            nc.gpsimd.dma_start(out=bv_b, in_=bcast(bv_h[:]))
            eps_sb = consts.tile([128, 1], F32)
            nc.vector.memset(eps_sb, LN_EPS)
            warm = consts.tile([128, 1], F32)
            nc.scalar.activation(out=warm, in_=eps_sb, func=AF.Exp)
            ones_c = consts.tile([128, 1], F32)
            nc.vector.memset(ones_c, 1.0)
            ident = consts.tile([128, 128], F32)
            make_identity(nc, ident)
            identb = consts.tile([128, 128], BF16)
            nc.vector.tensor_copy(identb, ident)

            for _rep in range(reps):
                v_sb = []
                with tc.tile_pool(name="et", bufs=8) as etp:
                    et = []
                    for k in range(8):
                        t = etp.tile([128, S], F32R, name="et", tag="et")
                        getattr(nc, ENC_Q[k]).dma_start(
                            out=t, in_=encT_h[k * 128:(k + 1) * 128, :])
                        et.append(t)

                    with (
                        tc.tile_pool(name="wkq", bufs=1) as wkq,
                        tc.tile_pool(name="dt", bufs=8) as dtp,
                    ):
                        wk_sb = wkq.tile([128, 8, DG], F32R, tag="wk")
                        nc.sync.dma_start(out=wk_sb, in_=wkT_h[:].rearrange("(t p) n -> p t n", p=128))
                        dt_ = []
                        for k in range(8):
                            t = dtp.tile([128, S], F32R, name="dt", tag="dt")
                            getattr(nc, DEC_Q[k]).dma_start(
                                out=t, in_=decT_h[k * 128:(k + 1) * 128, :])
                            dt_.append(t)
                        wv_sb = wkq.tile([128, 8, DG], F32R, tag="wv")
                        nc.scalar.dma_start(out=wv_sb, in_=wvT_h[:].rearrange("(t p) n -> p t n", p=128))
                        wq_sb = wkq.tile([128, 8, DG], F32R, tag="wq")
                        nc.scalar.dma_start(out=wq_sb, in_=wqT_h[:].rearrange("(t p) n -> p t n", p=128))

                        # K projection: k-outer over 8 live psums, streams with encT
                        with tc.tile_pool(name="pk8", bufs=1, space="PSUM") as pk8:
                            kps = [pk8.tile([128, 512], F32, name=f"kps{mn}", tag=f"kps{mn}")
                                   for mn in range(8)]
                            for k in range(8):
                                for mn in range(8):
                                    m, n = mn // 4, mn % 4
                                    nc.tensor.matmul(
                                        kps[mn],
                                        lhsT=wk_sb[:, k, m * 128:(m + 1) * 128],
                                        rhs=et[k][:, n * 512:(n + 1) * 512],
                                        start=(k == 0),
                                        stop=(k == 7),
                                        skip_group_check=True,
                                    )
                            for mn in range(8):
                                m, n = mn // 4, mn % 4
                                if mn % 2 == 0:
                                    nc.vector.tensor_scalar_add(
                                        kT[m][:, n * 512:(n + 1) * 512], kps[mn], bk_sb[:, m:m + 1]
                                    )
                                else:
                                    nc.scalar.activation(
                                        out=kT[m][:, n * 512:(n + 1) * 512], in_=kps[mn],
                                        func=AF.Identity, bias=bk_sb[:, m:m + 1],
                                    )

                        # V projection (encT resident, before Q)
                        with tc.tile_pool(name="pjv", bufs=2, space="PSUM") as pjv:
                            for i in range(16):
                                ps = pjv.tile([128, DG], F32, tag="pv")
                                for k in range(8):
                                    nc.tensor.matmul(
                                        ps,
                                        lhsT=et[k][:, i * 128:(i + 1) * 128],
                                        rhs=wv_sb[:, k, :],
                                        start=(k == 0),
                                        stop=(k == 7),
                                    )
                                vt = vpool.tile([128, HPC * 65], BF16, tag="v")
                                oc = ones_c[:]
                                oc4 = bass.AP(tensor=oc.tensor, offset=oc.offset,
                                              ap=[list(oc.ap[0]), [0, 4]])
                                nc.vector.tensor_copy(vt[:, 64:HPC * 65:65], oc4)
                                vt3 = bass.AP(tensor=vt.tensor, offset=vt.offset,
                                              ap=[list(vt.ap[0]), [65, 4], [1, 64]])
                                ps3 = bass.AP(tensor=ps.tensor, offset=ps.offset,
                                              ap=[list(ps.ap[0]), [64, 4], [1, 64]])
                                bv3 = bass.AP(tensor=bv_b.tensor, offset=bv_b.offset,
                                              ap=[list(bv_b.ap[0]), [64, 4], [1, 64]])
                                nc.vector.tensor_add(vt3, ps3, bv3)
                                v_sb.append(vt)

                        # Q projection: n-outer, k-inner (2 psum bufs ping-pong)
                        with tc.tile_pool(name="pq", bufs=2, space="PSUM") as pq:
                            for n in range(4):
                                qp = pq.tile([128, 2, 512], F32, tag="qp")
                                for m in range(2):
                                    for k in range(8):
                                        nc.tensor.matmul(
                                            qp[:, m, :],
                                            lhsT=wq_sb[:, k, m * 128:(m + 1) * 128],
                                            rhs=dt_[k][:, n * 512:(n + 1) * 512],
                                            start=(k == 0),
                                            stop=(k == 7),
                                            skip_group_check=True,
                                        )
                                nc.vector.tensor_scalar_add(
                                    qT[0][:, n * 512:(n + 1) * 512], qp[:, 0, :], bq_sb[:, 0:1]
                                )
                                nc.scalar.activation(
                                    out=qT[1][:, n * 512:(n + 1) * 512], in_=qp[:, 1, :],
                                    func=AF.Identity, bias=bq_sb[:, 1:2],
                                )

                    # ---------------- attention ----------------
                    with (
                        tc.tile_pool(name="sps", bufs=5, space="PSUM") as sps,
                        tc.tile_pool(name="pvps", bufs=1, space="PSUM") as pvps,
                        tc.tile_pool(name="tps", bufs=1, space="PSUM") as tps,
                        tc.tile_pool(name="exps", bufs=4) as exps,
                        tc.tile_pool(name="ztp", bufs=2) as ztp,
                        tc.tile_pool(name="zfp", bufs=2) as zfp,
                        tc.tile_pool(name="xc", bufs=2) as xc,
                        tc.tile_pool(name="sm", bufs=8) as sm,
                    ):
                        for hp in range(2):
                            xts = [xc.tile([128, D], F32, tag="x", name=f"xt{hp}_{hl}")
                                   for hl in range(2)]
                            dcts = [xc.tile([128, D], F32, tag="dc", name=f"dct{hp}_{hl}")
                                    for hl in range(2)]
                            for hl in range(2):
                                h = hp * 2 + hl
                                nc.sync.dma_start(out=dcts[hl], in_=dec_blk_h[h * 128:(h + 1) * 128, :])
                            for j in range(4):
                                pvs = [pvps.tile([65, 512], F32, name=f"pv{hl}", tag=f"pv{hl}")
                                       for hl in range(2)]
                                exs = {}

                                def s_step(g):
                                    ex = exps.tile([128, 2, 512], BF16, tag="ex")
                                    for hl in range(2):
                                        sp = sps.tile([128, 512], F32, tag="s")
                                        nc.tensor.matmul(
                                            sp,
                                            lhsT=kT[hp][hl * 64:hl * 64 + 64, g * 128:(g + 1) * 128],
                                            rhs=qT[hp][hl * 64:hl * 64 + 64, j * 512:(j + 1) * 512],
                                            start=True,
                                            stop=True,
                                        )
                                        sel = EXP_PAT[(g * 2 + hl) % 32]
                                        if sel == "A":
                                            nc.scalar.activation(
                                                out=ex[:, hl, :], in_=sp, func=AF.Exp)
                                        else:
                                            nc.vector.tensor_scalar(
                                                out=ex[:, hl, :].bitcast(I16),
                                                in0=sp,
                                                scalar1=A16, scalar2=B16,
                                                op0=OP.mult, op1=OP.add,
                                            )
                                    exs[g] = ex

                                def pv_step(g):
                                    ex = exs.pop(g)
                                    for hl in range(2):
                                        nc.tensor.matmul(
                                            pvs[hl],
                                            lhsT=v_sb[g][:, (hp * 2 + hl) * 65:(hp * 2 + hl) * 65 + 65],
                                            rhs=ex[:, hl, :],
                                            start=(g == 0),
                                            stop=(g == 15),
                                            skip_group_check=True,
                                        )

                                s_step(0)
                                for g in range(1, 16):
                                    s_step(g)
                                    pv_step(g - 1)
                                pv_step(15)

                                # epilogue: z^T pair -> transpose -> scale -> HBM
                                for hl in range(2):
                                    h = hp * 2 + hl
                                    zt = ztp.tile([128, 512], BF16, tag="zt")
                                    nc.scalar.activation(out=zt[0:65, :], in_=pvs[hl], func=AF.Copy)
                                    tp4 = tps.tile([128, 4, 66], BF16, tag="tp")
                                    for qq in range(4):
                                        nc.tensor.transpose(
                                            out=tp4[:, qq, 0:65],
                                            in_=zt[0:65, qq * 128:(qq + 1) * 128],
                                            identity=identb[0:65, 0:65],
                                        )
                                    rc = sm.tile([128, 4, 1], F32, tag="rc")
                                    nc.vector.reciprocal(rc, tp4[:, :, 64:65])
                                    rcb = bass.AP(tensor=rc.tensor, offset=rc.offset,
                                                  ap=[list(rc.ap[0]), [1, 4], [0, 64]])
                                    zf = zfp.tile([128, 4, 64], F32, tag="zf")
                                    nc.vector.tensor_mul(zf, tp4[:, :, 0:64], rcb)
                                    nc.sync.dma_start(
                                        out=zd[h][j * 512:(j + 1) * 512, :].rearrange(
                                            "(q p) d -> p q d", p=128),
                                        in_=zf,
                                    )
                                    nc.sync.dma_start(
                                        out=xts[hl][j * 32:(j + 1) * 32, :],
                                        in_=zd[h][j * 512:(j + 1) * 512, :].rearrange(
                                            "(u w) d -> u (w d)", w=16),
                                    )
                                    sl = slice(j * 32, (j + 1) * 32)
                                    eng = nc.vector if hl == 0 else nc.gpsimd
                                    eng.tensor_add(xts[hl][sl, :], xts[hl][sl, :], dcts[hl][sl, :])

                            # ---- LayerNorm for this head pair (paired chain) ----
                            sts = []
                            for hl in range(2):
                                st = sm.tile([128, 2, 6], F32, tag=f"st{hl}")
                                for s2 in range(2):
                                    nc.vector.bn_stats(out=st[:, s2, :], in_=xts[hl][:, s2 * 512:(s2 + 1) * 512])
                                sts.append(st)
                            mv2 = sm.tile([128, 2, 2], F32, tag="mv2")
                            for hl in range(2):
                                nc.vector.bn_aggr(out=mv2[:, hl, :], in_=sts[hl])
                            vv2 = sm.tile([128, 2], F32, tag="vv2")
                            nc.vector.tensor_scalar_add(vv2, mv2[:, :, 1], LN_EPS)
                            ti2 = sm.tile([128, 2], mybir.dt.int32, tag="ti2")
                            nc.vector.tensor_scalar(
                                out=ti2, in0=vv2[:].bitcast(mybir.dt.int32), scalar1=1,
                                scalar2=None, op0=OP.logical_shift_right,
                            )
                            nc.vector.tensor_scalar(
                                out=ti2, in0=ti2, scalar1=-1, scalar2=0x5F3759DF,
                                op0=OP.mult, op1=OP.add,
                            )
                            y2 = sm.tile([128, 2], F32, tag="y2")
                            nc.vector.tensor_copy(y2, ti2[:].bitcast(F32))
                            t2 = sm.tile([128, 2], F32, tag="t2")
                            for _ in range(2):
                                nc.vector.tensor_mul(t2, vv2, y2)
                                nc.vector.tensor_mul(t2, t2, y2)
                                nc.vector.tensor_scalar(
                                    out=t2, in0=t2, scalar1=-0.5, scalar2=1.5,
                                    op0=OP.mult, op1=OP.add,
                                )
                                nc.vector.tensor_mul(y2, y2, t2)
                            for hl in range(2):
                                h = hp * 2 + hl
                                xn = xc.tile([128, D], F32, tag="xn", name=f"xn{hp}_{hl}")
                                nc.vector.tensor_scalar(
                                    out=xn, in0=xts[hl], scalar1=mv2[:, hl, 0:1], scalar2=y2[:, hl:hl + 1],
                                    op0=OP.subtract, op1=OP.mult,
                                )
                                nc.sync.dma_start(out=out_h[h * 128:(h + 1) * 128, :], in_=xn)

    nc.compile()
    return nc


_NC_CACHE = None


def _get_nc():
    global _NC_CACHE
    if _NC_CACHE is None:
        _NC_CACHE = build_bass()
    return _NC_CACHE


def make_in_maps(encoded, decoded, Wq, bq, Wk, bk, Wv, bv, gamma, beta):
    encoded = np.asarray(encoded, dtype=np.float32)
    decoded = np.asarray(decoded, dtype=np.float32)
    Wq, bq = np.asarray(Wq, np.float32), np.asarray(bq, np.float32)
    Wk, bk = np.asarray(Wk, np.float32), np.asarray(bk, np.float32)
    Wv, bv = np.asarray(Wv, np.float32), np.asarray(bv, np.float32)

    encT = [np.ascontiguousarray(encoded[b].T) for b in range(B)]
    decT = [np.ascontiguousarray(decoded[b].T) for b in range(B)]
    in_maps = []
    for c in range(N_CORES):
        b, hg = c // HPC, c % HPC
        sl = slice(hg * DG, (hg + 1) * DG)
        rows = slice(hg * ROWS, (hg + 1) * ROWS)
        in_maps.append({
            "encT": encT[b],
            "decT": decT[b],
            "wqT": np.ascontiguousarray(Wq[sl, :].T),
            "wkT": np.ascontiguousarray(Wk[sl, :].T),
            "wvT": np.ascontiguousarray(Wv[sl, :].T),
            "bq": np.ascontiguousarray(bq[sl]),
            "bk": np.ascontiguousarray(bk[sl]),
            "bv": np.ascontiguousarray(bv[sl]),
            "dec_blk": np.ascontiguousarray(decoded[b, rows]),
        })
    return in_maps


def kernel(**inputs) -> np.ndarray:
    nc = _get_nc()
    in_maps = make_in_maps(**inputs)
    res = run_bass_kernel_spmd(nc, in_maps, list(range(N_CORES)))
    out = np.empty((B, S, D), dtype=np.float32)
    for c in range(N_CORES):
        b, hg = c // HPC, c % HPC
        out[b, hg * ROWS:(hg + 1) * ROWS, :] = res.results[c]["out"]
    # gamma/beta are identity under the reference's setup_inputs(); apply
    # host-side only if they deviate.
    gamma = np.asarray(inputs["gamma"], np.float32)
    beta = np.asarray(inputs["beta"], np.float32)
    if not (np.all(gamma == 1.0) and np.all(beta == 0.0)):
        out = out * gamma + beta
    return out
